# revision 11
# baseline (speedup 1.0000x reference)
"""Causal self-attention (B=4, T=2048, C=768, 12 heads) on 8 Trainium2 NeuronCores.

Sharding: core c -> batch b = c//2, head group hg = c%2 (6 heads each).
Each core computes QKV for its (b, 6 heads), flash-style causal attention in a
transposed layout (scores^T = K^T-chunk @ Q, exp on ACT, ones-column in V for
row sums via the PE), transposes the per-head attention output on the PE, and
applies its slice of the output projection. Host sums the two half-head partial
y's per batch and adds b_proj.

Schedule: hp-major attention pipeline. Attention scores for head-pair hp start
as soon as q/k for hp0 token-chunk 0 exist; the remaining QKV matmuls (other
q/k chunks + all of V) are issued as PE fillers interleaved into the EXP-paced
scores loop so the PE never starves while ACT works through the exps. AV for
(hp,qt) lags one step behind scores (lag-1 software pipeline across hp
boundaries); projection chases AV during hp2. Diagonal-supertile score matmuls
are trimmed to the causal region.
"""

import numpy as np

import concourse.bacc as bacc
import concourse.bass as bass
import concourse.mybir as mybir
import concourse.tile as tile

# problem constants (hardcoded per harness contract)
B, T, C = 4, 2048, 768
NH, HD = 12, 64
NHL = 6            # heads per core
N_CORES = 8
TQ = 512           # q supertile width
NTB = T // 128     # 16 token blocks

F32 = mybir.dt.float32


def _build_program(cdt=mybir.dt.bfloat16, n_iters=1, with_bqk=False, with_bv=False):
    """Build the SPMD single-core program. Returns nc."""
    nc = bacc.Bacc("TRN2", target_bir_lowering=False, debug=False,
                   num_devices=N_CORES)

    xT_d = nc.dram_tensor("xT", [C, T], cdt, kind="ExternalInput")
    wqk_d = nc.dram_tensor("wqk", [C, 768], cdt, kind="ExternalInput")
    wv_d = nc.dram_tensor("wv", [C, 384], cdt, kind="ExternalInput")
    wp_d = nc.dram_tensor("wp", [384, C], cdt, kind="ExternalInput")
    mask_d = nc.dram_tensor("mask", [128, 128], cdt, kind="ExternalInput")
    id_d = nc.dram_tensor("ident", [128, 128], cdt, kind="ExternalInput")
    if with_bqk:
        bqk_d = nc.dram_tensor("bqk", [C, 1], F32, kind="ExternalInput")
    if with_bv:
        bv_d = nc.dram_tensor("bv", [128, 384], cdt, kind="ExternalInput")
    y_d = nc.dram_tensor("y", [T, C], F32, kind="ExternalOutput")

    with tile.TileContext(nc) as tc:
        with (
            tc.tile_pool(name="consts", bufs=1) as consts,
            tc.tile_pool(name="big", bufs=1) as big,
            tc.tile_pool(name="work", bufs=3) as work,
            tc.tile_pool(name="expool", bufs=30) as expool,
            tc.tile_pool(name="attp", bufs=3) as attp,
            tc.tile_pool(name="small", bufs=8) as small,
            tc.tile_pool(name="ps", bufs=3, space="PSUM") as psp,
        ):
            xT = consts.tile([128, 6, T], cdt)
            wqk = consts.tile([128, 6, 768], cdt)
            wv = consts.tile([128, 6, 384], cdt)
            wp = consts.tile([128, 3, 768], cdt)
            mask = consts.tile([128, 128], cdt)
            ident = consts.tile([128, 128], cdt)
            if with_bqk:
                bqk = consts.tile([128, 6, 1], F32)
            if with_bv:
                bv = consts.tile([128, 384], cdt)

            qt_sb = big.tile([128, 3, T], cdt)
            kt_sb = big.tile([128, 3, T], cdt)
            v_sb = big.tile([128, NTB, NHL, 65], cdt)
            attT_sb = big.tile([128, 3, T], cdt)

            def qk_nt(j, nt):
                """One 512-token chunk of q (j<3) / k (j>=3), partition chunk j%3."""
                dst = qt_sb if j < 3 else kt_sb
                jj = j % 3
                ps = psp.tile([128, 2, 512], F32, tag="A", name="qk")
                for kc in range(6):
                    nc.tensor.matmul(
                        ps[:, 0, :],
                        lhsT=wqk[:, kc, j * 128:(j + 1) * 128],
                        rhs=xT[:, kc, nt * 512:(nt + 1) * 512],
                        start=(kc == 0), stop=(kc == 5),
                    )
                if with_bqk:
                    nc.vector.tensor_scalar_add(
                        dst[:, jj, nt * 512:(nt + 1) * 512], ps[:, 0, :],
                        bqk[:, jj if j < 3 else jj + 3])
                else:
                    nc.vector.tensor_copy(
                        dst[:, jj, nt * 512:(nt + 1) * 512], ps[:, 0, :])

            def v_tb(tb):
                """V projection for one 128-token block, natural layout."""
                ps = psp.tile([128, 2, 512], F32, tag="A", name="psv")
                for kc in range(6):
                    nc.tensor.matmul(
                        ps[:, 0, 0:384],
                        lhsT=xT[:, kc, tb * 128:(tb + 1) * 128],
                        rhs=wv[:, kc, :],
                        start=(kc == 0), stop=(kc == 5),
                    )
                psr = ps[:, 0, 0:384].rearrange("p (h d) -> p h d", h=NHL)
                if with_bv:
                    nc.scalar.activation(
                        out=v_sb[:, tb, :, 0:HD], in_=psr,
                        func=mybir.ActivationFunctionType.Identity,
                        bias=0.0, scale=1.0)
                    nc.vector.tensor_add(
                        v_sb[:, tb, :, 0:HD], v_sb[:, tb, :, 0:HD],
                        bv.rearrange("p (h d) -> p h d", h=NHL))
                else:
                    nc.vector.tensor_copy(v_sb[:, tb, :, 0:HD], psr)

            def do_av(hp, qt, exs):
                """Normalized attention for (hp, qt) + transpose into attT_sb."""
                nkb = 4 * qt + 4
                att = attp.tile([128, 4, 2, HD], cdt, name="att")
                for h01 in range(2):
                    h = 2 * hp + h01
                    op = psp.tile([128, 4, 65], F32, tag="B", name="op", bufs=2)
                    for qq in range(4):
                        kbs = [kb for kb in range(nkb) if kb - 4 * qt <= qq]
                        for idx, kb in enumerate(kbs):
                            nc.tensor.matmul(
                                op[:, qq, :],
                                lhsT=exs[kb][:, h01, qq * 128:(qq + 1) * 128],
                                rhs=v_sb[:, kb, h, :],
                                start=(idx == 0), stop=(idx == len(kbs) - 1),
                            )
                    rc = small.tile([128, 4], F32, tag="rc")
                    nc.vector.reciprocal(rc[:], op[:, :, HD])
                    for qq in range(4):
                        nc.vector.tensor_scalar_mul(
                            att[:, qq, h01, :], op[:, qq, 0:HD], rc[:, qq:qq + 1])
                for qq in range(4):
                    qb = qt * 4 + qq
                    tp = psp.tile([128, 4, 65], cdt, tag="B", name="tp", bufs=2)
                    tpa = tp.rearrange("p a b -> p (a b)")[:, 0:128]
                    nc.tensor.transpose(tpa, att[:, qq, :, :], ident[:])
                    nc.vector.tensor_copy(
                        attT_sb[:, hp, qb * 128:(qb + 1) * 128], tpa)

            def do_proj(qt):
                """Output projection + store for qt's 4 token blocks."""
                for tb in range(qt * 4, qt * 4 + 4):
                    ps = psp.tile([128, 2, 512], F32, tag="A", name="pp")
                    for hp2 in range(3):
                        nc.tensor.matmul(
                            ps[:, 0, :],
                            lhsT=attT_sb[:, hp2, tb * 128:(tb + 1) * 128],
                            rhs=wp[:, hp2, 0:512],
                            start=(hp2 == 0), stop=(hp2 == 2))
                        nc.tensor.matmul(
                            ps[:, 1, 0:256],
                            lhsT=attT_sb[:, hp2, tb * 128:(tb + 1) * 128],
                            rhs=wp[:, hp2, 512:768],
                            start=(hp2 == 0), stop=(hp2 == 2))
                    ysb = work.tile([128, 768], F32, tag="ysb")
                    nc.vector.tensor_copy(ysb[:, 0:512], ps[:, 0, :])
                    nc.vector.tensor_copy(ysb[:, 512:768], ps[:, 1, 0:256])
                    nc.sync.dma_start(
                        y_d[tb * 128:(tb + 1) * 128, :], ysb[:])

            xT_r = xT_d.rearrange("(n p) t -> p n t", p=128)
            wqk_r = wqk_d.rearrange("(n p) m -> p n m", p=128)
            wv_r = wv_d.rearrange("(n p) m -> p n m", p=128)
            wp_r = wp_d.rearrange("(n p) m -> p n m", p=128)

            def load_main_inputs():
                for kc in range(6):
                    nc.sync.dma_start(xT[:, kc, :], xT_r[:, kc, :])
                    nc.sync.dma_start(wqk[:, kc, :], wqk_r[:, kc, :])
                for kc in range(6):
                    nc.sync.dma_start(wv[:, kc, :], wv_r[:, kc, :])
                if with_bqk:
                    nc.sync.dma_start(bqk[:], bqk_d.rearrange("(n p) o -> p n o", p=128))
                if with_bv:
                    nc.sync.dma_start(bv[:], bv_d[:])

            def prologue():
                load_main_inputs()
                nc.sync.dma_start(mask[:], mask_d[:])
                nc.sync.dma_start(ident[:], id_d[:])
                nc.sync.dma_start(wp[:], wp_r)
                nc.gpsimd.memset(v_sb[:, :, :, HD:HD + 1], 1.0)

            def body(reload=False):
                # lead-in: q/k for head-pair 0, token chunk 0 only
                qk_nt(0, 0)
                qk_nt(3, 0)

                # PE filler work, interleaved into the scores loop one item
                # per kb-block. hp0 carries the work that must land before
                # hp1 (q0/k0 tail chunks, all of V, q1/k1); hp1 carries
                # q2/k2 (needed by hp2 scores) paced evenly so the PE keeps
                # feeding during the ACT-bound middle of the iteration.
                hp_fillers = {0: [], 1: [], 2: []}
                hp_fillers[0] += [("qk", 0, 1), ("qk", 3, 1),
                                  ("v", 0), ("v", 1), ("v", 2), ("v", 3),
                                  ("qk", 0, 2), ("qk", 3, 2),
                                  ("qk", 0, 3), ("qk", 3, 3)]
                for tb in range(4, NTB):
                    hp_fillers[0].append(("v", tb))
                for j in (1, 4):
                    for nt in range(4):
                        hp_fillers[0].append(("qk", j, nt))
                for j in (2, 5):
                    for nt in range(4):
                        hp_fillers[1].append(("qk", j, nt))
                if reload:
                    # Re-issue next iteration's input loads right after the
                    # last reads of xT/wqk/wv. The SP stream stays ahead of
                    # the y-output DMAs so reloads overlap the attention tail
                    # instead of stalling the next iteration's start.
                    hp_fillers[1].append(("reload",))
                fill_state = {hp: [0, 0] for hp in range(3)}  # [popped, slot]

                def pop_filler(hp):
                    """Advance one kb slot for this hp and issue due fillers.
                    hp0: one per slot (dependency-ordered: V blocks must land
                    before their AV, q/k chunks before their scores). hp1:
                    paced evenly over its 40 kb slots."""
                    fl = hp_fillers[hp]
                    st = fill_state[hp]
                    st[1] += 1
                    if hp == 0:
                        want = min(len(fl), st[1])
                    else:
                        want = min(len(fl), (st[1] * len(fl) + 39) // 40)
                    while st[0] < want:
                        f = fl[st[0]]
                        st[0] += 1
                        if f[0] == "qk":
                            qk_nt(f[1], f[2])
                        elif f[0] == "v":
                            v_tb(f[1])
                        else:
                            load_main_inputs()

                seq = [(hp, qt) for hp in range(3) for qt in range(4)]
                prev = None          # (hp, qt, exs) pending AV
                for hp, qt in seq:
                    nkb = 4 * qt + 4
                    exs = []
                    for kb in range(nkb):
                        diag = kb - 4 * qt
                        q_lo = diag * 128 if diag >= 0 else 0
                        sc = psp.tile([128, 2, 512], F32, tag="A", name="sc")
                        for h01 in range(2):
                            pb = h01 * 64
                            nc.tensor.matmul(
                                sc[:, h01, q_lo:512],
                                lhsT=kt_sb[pb:pb + 64, hp, kb * 128:(kb + 1) * 128],
                                rhs=qt_sb[pb:pb + 64, hp, qt * 512 + q_lo:(qt + 1) * 512],
                                start=True, stop=True,
                            )
                        ex = expool.tile([128, 2, 512], cdt, tag="ex", name="ex")
                        nc.scalar.activation(
                            out=ex[:, :, q_lo:512], in_=sc[:, :, q_lo:512],
                            func=mybir.ActivationFunctionType.Exp)
                        if diag >= 0:
                            for h01 in range(2):
                                nc.gpsimd.tensor_mul(
                                    ex[:, h01, q_lo:q_lo + 128],
                                    ex[:, h01, q_lo:q_lo + 128],
                                    mask[:])
                        exs.append(ex)
                        pop_filler(hp)
                    if prev is not None:
                        phh, pqt, pexs = prev
                        do_av(phh, pqt, pexs)
                        if phh == 2:
                            do_proj(pqt)
                    prev = (hp, qt, exs)
                # drain any un-issued fillers (safety net; normally empty)
                for hp in range(3):
                    while fill_state[hp][0] < len(hp_fillers[hp]):
                        pop_filler(hp)
                phh, pqt, pexs = prev
                do_av(phh, pqt, pexs)
                do_proj(pqt)
                if reload:
                    # wp is only dead after the final proj; reload it behind
                    # the y-output DMAs so they are not held up.
                    nc.sync.dma_start(wp[:], wp_r)

            prologue()
            if n_iters == 1:
                body(reload=False)
            else:
                with tc.For_i(0, n_iters, 1,
                              staggered_reset=True,
                              hint_engines=(mybir.EngineType.PE,
                                            mybir.EngineType.DVE,
                                            mybir.EngineType.Activation)):
                    body(reload=True)

    nc.compile()
    return nc


def _host_prep(inputs, cdt_np):
    """Per-core input maps from full inputs."""
    x = np.asarray(inputs["x"], np.float32)
    w_attn = np.asarray(inputs["w_attn"], np.float32)
    b_attn = np.asarray(inputs["b_attn"], np.float32)
    w_proj = np.asarray(inputs["w_proj"], np.float32)

    mask = (np.arange(128)[:, None] <= np.arange(128)[None, :]).astype(cdt_np)
    ident = np.eye(128, dtype=cdt_np)
    with_bqk = bool(np.any(b_attn[0:1536] != 0))
    with_bv = bool(np.any(b_attn[1536:2304] != 0))

    in_maps = []
    for c in range(N_CORES):
        b, hg = c // 2, c % 2
        cols = slice(hg * 384, hg * 384 + 384)
        wq = w_attn[:, 0:768][:, cols] * 0.125
        wk = w_attn[:, 768:1536][:, cols]
        m = {
            "xT": np.ascontiguousarray(x[b].T).astype(cdt_np),
            "wqk": np.concatenate([wq, wk], axis=1).astype(cdt_np),
            "wv": np.ascontiguousarray(w_attn[:, 1536:2304][:, cols]).astype(cdt_np),
            "wp": np.ascontiguousarray(w_proj[cols, :]).astype(cdt_np),
            "mask": mask,
            "ident": ident,
        }
        if with_bqk:
            bq = b_attn[0:768][cols] * 0.125
            bk = b_attn[768:1536][cols]
            m["bqk"] = np.concatenate([bq, bk]).astype(np.float32).reshape(C, 1)
        if with_bv:
            bv = b_attn[1536:2304][cols].astype(cdt_np)
            m["bv"] = np.broadcast_to(bv, (128, 384)).copy()
        in_maps.append(m)
    return in_maps, with_bqk, with_bv


_CACHE = {}


def _get_runner(cdt, n_iters, with_bqk, with_bv, donate=True):
    """Build program + persistent jitted PJRT callable (cached)."""
    key = (str(cdt), n_iters, with_bqk, with_bv, donate)
    if key in _CACHE:
        return _CACHE[key]

    import jax
    from jax.sharding import Mesh, PartitionSpec
    from jax.experimental.shard_map import shard_map
    from concourse.bass2jax import (_bass_exec_p, install_neuronx_cc_hook,
                                    partition_id_tensor)

    nc = _build_program(cdt=cdt, n_iters=n_iters,
                        with_bqk=with_bqk, with_bv=with_bv)
    install_neuronx_cc_hook()

    partition_name = nc.partition_id_tensor.name if nc.partition_id_tensor else None
    in_names, out_names, out_avals = [], [], []
    for alloc in nc.m.functions[0].allocations:
        if not isinstance(alloc, mybir.MemoryLocationSet):
            continue
        name = alloc.memorylocations[0].name
        if alloc.kind == "ExternalInput":
            if name != partition_name:
                in_names.append(name)
        elif alloc.kind == "ExternalOutput":
            out_names.append(name)
            out_avals.append(jax.core.ShapedArray(
                tuple(alloc.tensor_shape), mybir.dt.np(alloc.dtype)))
    n_params = len(in_names)
    n_outs = len(out_avals)
    all_names = list(in_names) + list(out_names)
    if partition_name is not None:
        all_names.append(partition_name)
    donate_ = tuple(range(n_params, n_params + n_outs))

    def _bodyfn(*args):
        operands = list(args)
        if partition_name is not None:
            operands.append(partition_id_tensor())
        outs = _bass_exec_p.bind(
            *operands,
            out_avals=tuple(out_avals),
            in_names=tuple(all_names),
            out_names=tuple(out_names),
            lowering_input_output_aliases=(),
            sim_require_finite=True,
            sim_require_nnan=True,
            nc=nc,
        )
        return tuple(outs)

    devices = jax.devices()[:N_CORES]
    mesh = Mesh(np.asarray(devices), ("core",))
    in_specs = (PartitionSpec("core"),) * (n_params + n_outs)
    out_specs = (PartitionSpec("core"),) * n_outs
    fn = jax.jit(
        shard_map(_bodyfn, mesh=mesh, in_specs=in_specs, out_specs=out_specs,
                  check_rep=False),
        donate_argnums=donate_ if donate else (), keep_unused=True)

    runner = (fn, in_names, out_names, out_avals)
    _CACHE[key] = runner
    return runner


def _run(in_maps, cdt, n_iters, with_bqk, with_bv):
    import jax
    fn, in_names, out_names, out_avals = _get_runner(cdt, n_iters, with_bqk, with_bv)
    concat_in = [np.concatenate([m[nm] for m in in_maps], axis=0)
                 for nm in in_names]
    zeros = [np.zeros((N_CORES * av.shape[0], *av.shape[1:]), av.dtype)
             for av in out_avals]
    outs = fn(*concat_in, *zeros)
    jax.block_until_ready(outs)
    y = np.asarray(outs[out_names.index("y")]).reshape(N_CORES, T, C)
    return y


def kernel(**inputs) -> np.ndarray:
    import ml_dtypes
    cdt, cdt_np = mybir.dt.bfloat16, ml_dtypes.bfloat16
    in_maps, with_bqk, with_bv = _host_prep(inputs, cdt_np)
    y_parts = _run(in_maps, cdt, 1, with_bqk, with_bv)

    b_proj = np.asarray(inputs["b_proj"], np.float32)
    out = np.empty((B, T, C), np.float32)
    for b in range(B):
        out[b] = y_parts[2 * b] + y_parts[2 * b + 1] + b_proj
    return out
